# revision 24
# baseline (speedup 1.0000x reference)
"""Trainium2 Bass kernel for nn_Decoder_74577812127757.

Math (from the reference):
  - LSTM state (h0, c0) is NEVER updated across the 32 steps: the recurrent
    contribution hh = h0 @ W_hh.T + b_ih + b_hh and the cell input c0 are
    constants.
  - The embed->input-proj chain collapses:  x_t @ W_ih.T
        = rel_{t-1} @ Wc.T + speed * wz + W_ih @ b_emb
    with Wc = W_ih @ W_emb[:, :2]  (rank-2 per-step update!), wz = W_ih @ W_emb[:, 2].
  - last_pos / last_speed_abs_pos never affect the output; output #2 is h0[0:1].

Device strategy (pure data parallel over 8 cores, 8192 batch rows each):
  - Layout B: hidden/gate dims on partitions, batch on the free dim
    (long ACT instructions; ACT sigmoid/tanh dominates the roofline).
  - gates [128, 4*512] live in PSUM (4 banks) per 512-batch group:
    chunks [i|f|o|g].  i/f/o accumulate incrementally across steps via
    +Wc@rel_t - Wc@rel_{t-1} (one K=4 f32r matmul per chunk per step);
    the g bank is rebuilt each step (identity@base_g + Wc_g@rel) and then
    timeshared as the [2, 512] rel-matmul output.  2 groups in flight = all
    8 PSUM banks.
  - ACT: sigmoid over [0:1536] (one op), tanh(g), tanh(c_new).
  - DVE: 4 elementwise muls/adds (optionally bf16 for 2x mode) + the
    rel psum->sbuf copy (fused +b_pos via tensor_scalar_add).
  - Output written as relsT [T, 2, BC] (contiguous 2KB runs), transposed to
    [T, B, 2] on the host.
"""

import sys

if "/opt/trn_rl_repo" not in sys.path:
    sys.path.insert(0, "/opt/trn_rl_repo")

import numpy as np

import concourse.bass as bass
from concourse import bacc
import concourse.mybir as mybir
from concourse.tile import TileContext
from concourse.masks import make_identity
from concourse.bass_utils import run_bass_kernel_spmd

NCORES = 8
B = 65536
BC = B // NCORES  # 8192 batch rows per core
H = 128
G4 = 4 * H  # 512
T = 32
BF = 512  # batch rows per group
F32 = mybir.dt.float32
F32R = mybir.dt.float32r
BF16 = mybir.dt.bfloat16

# dtype of the activation outputs / DVE elementwise chain.
ACT_DT = BF16

AF = mybir.ActivationFunctionType


def _r(ap):
    """View an fp32 AP as float32r so matmuls run at 1 cycle/row (N>=256)."""
    return ap.bitcast(F32R)


OPTS = dict(step_bufs=6, grp_bufs=4, sigma_split=False, fuse_relr=True, pair_tanh=False, sigma_late=True, tanh_all=False)


def build_nc(bc=BC, act_dt=ACT_DT, **opts):
    o = dict(OPTS); o.update(opts)
    ng = bc // BF  # number of 512-row groups
    nc = bacc.Bacc("TRN2", target_bir_lowering=False)

    h0T_d = nc.dram_tensor("h0T", [H, bc], F32, kind="ExternalInput")
    c0T_d = nc.dram_tensor("c0T", [H, bc], F32, kind="ExternalInput")
    speed_d = nc.dram_tensor("speed", [1, bc], F32, kind="ExternalInput")
    lprelT_d = nc.dram_tensor("lprelT", [2, bc], F32, kind="ExternalInput")
    whhT_d = nc.dram_tensor("WhhT", [H, G4], F32, kind="ExternalInput")
    # column blocks: [0:512] = +Wc rows [x;y], [512:1024] = -Wc
    w4_d = nc.dram_tensor("W4", [2, 2 * G4], F32, kind="ExternalInput")
    init2_d = nc.dram_tensor("init2", [2, G4], F32, kind="ExternalInput")
    wposT_d = nc.dram_tensor("WposT", [H, 2], F32, kind="ExternalInput")
    bpos_d = nc.dram_tensor("bpos", [2, 1], F32, kind="ExternalInput")
    relsT_d = nc.dram_tensor("relsT", [T, 2, bc], F32, kind="ExternalOutput")

    with TileContext(nc) as tc:
        with (
            tc.tile_pool(name="const", bufs=1) as constp,
            tc.tile_pool(name="grp", bufs=o["grp_bufs"]) as grp,
            tc.tile_pool(name="stepp", bufs=o["step_bufs"]) as stepp,
            tc.tile_pool(name="gatesp", bufs=2, space="PSUM") as gatesp,
        ):
            whh_sb = constp.tile([H, G4], F32R, tag="whh")
            nc.gpsimd.dma_start(whh_sb, whhT_d[:])
            w4_sb = constp.tile([2, 2 * G4], F32R, tag="w4")
            nc.gpsimd.dma_start(w4_sb, w4_d[:])
            init2_sb = constp.tile([2, G4], F32R, tag="init2")
            nc.gpsimd.dma_start(init2_sb, init2_d[:])
            wpos_f32 = constp.tile([H, 2], F32, tag="wposf")
            nc.sync.dma_start(wpos_f32, wposT_d[:])
            bpos_sb = constp.tile([2, 1], F32, tag="bpos")
            nc.sync.dma_start(bpos_sb, bpos_d[:])
            ident_f = constp.tile([H, H], F32, tag="identf")
            make_identity(nc, ident_f)
            ident = constp.tile([H, H], F32R, tag="ident")
            nc.vector.tensor_copy(ident, ident_f)
            wpos_dt = act_dt if act_dt != F32 else F32R
            wpos_sb = constp.tile([H, 2], wpos_dt, tag="wposc")
            nc.vector.tensor_copy(wpos_sb, wpos_f32)

            def setup(g):
                b0 = g * BF
                st = {"b0": b0}
                h0g = grp.tile([H, BF], F32R, tag="h0g")
                nc.gpsimd.dma_start(h0g, h0T_d[:, b0 : b0 + BF])
                c0g = grp.tile([H, BF], act_dt, tag="c0g")
                if act_dt != F32:
                    nc.gpsimd.dma_start(c0g, c0T_d[:, b0 : b0 + BF])
                else:
                    nc.sync.dma_start(c0g, c0T_d[:, b0 : b0 + BF])
                st["c0g"] = c0g
                ping0 = grp.tile([2, BF], F32, tag="ping0")
                ping1 = grp.tile([2, BF], F32, tag="ping1")
                nc.sync.dma_start(ping1, lprelT_d[:, b0 : b0 + BF])
                pingr0 = grp.tile([2, BF], F32R, tag="pingr0")
                pingr1 = grp.tile([2, BF], F32R, tag="pingr1")
                nc.vector.tensor_copy(pingr1, ping1)
                st["ping"] = [ping0, ping1]
                st["pingr"] = [pingr0, pingr1]
                rhs2_f = grp.tile([2, BF], F32, tag="rhs2f")
                nc.vector.memset(rhs2_f, 1.0)
                nc.sync.dma_start(rhs2_f[0:1, :], speed_d[:, b0 : b0 + BF])
                rhs2 = grp.tile([2, BF], F32R, tag="rhs2")
                nc.vector.tensor_copy(rhs2, rhs2_f)

                gates = gatesp.tile([H, 4 * BF], F32, tag="gates")
                st["gates"] = gates
                for c in range(4):
                    dst = gates[:, c * BF : (c + 1) * BF]
                    nc.tensor.matmul(
                        dst,
                        whh_sb[:, c * H : (c + 1) * H],
                        h0g,
                        start=True,
                        stop=False,
                        skip_group_check=True,
                    )
                    nc.tensor.matmul(
                        dst,
                        init2_sb[:, c * H : (c + 1) * H],
                        rhs2,
                        start=False,
                        stop=False,
                        skip_group_check=True,
                    )
                    if c < 3:
                        # fold W2 @ last_pos_rel into the initial i/f/o state
                        nc.tensor.matmul(
                            dst,
                            w4_sb[:, c * H : (c + 1) * H],
                            pingr1,
                            start=False,
                            stop=False,
                            skip_group_check=True,
                        )
                base_g = grp.tile([H, BF], F32R, tag="base_g")
                nc.vector.tensor_copy(base_g, gates[:, 3 * BF : 4 * BF])
                st["base_g"] = base_g
                return st

            def step(st, t, cn_out):
                gates = st["gates"]
                b0 = st["b0"]
                cur_tile = st["ping"][t % 2]  # rel_t written here
                cur_r = st["pingr"][t % 2]
                prev_r = st["pingr"][(t - 1) % 2]  # holds rel_{t-1} (f32r)
                gdst = gates[:, 3 * BF : 4 * BF]
                if t >= 1:
                    for c in range(3):
                        # +Wc @ rel_{t-1} (fresh; the -Wc @ rel_{t-2} half was
                        # already emitted at the end of step t-1)
                        nc.tensor.matmul(
                            gates[:, c * BF : (c + 1) * BF],
                            w4_sb[:, c * H : (c + 1) * H],
                            prev_r,
                            start=False,
                            stop=False,
                            skip_group_check=True,
                        )
                    nc.tensor.matmul(
                        gdst,
                        ident,
                        st["base_g"],
                        start=True,
                        stop=False,
                        skip_group_check=True,
                    )
                nc.tensor.matmul(
                    gdst,
                    w4_sb[:, 3 * H : 4 * H],
                    prev_r,
                    start=False,
                    stop=True,
                    skip_group_check=True,
                )
                if o["tanh_all"]:
                    # gates for i/f/o were accumulated PRE-HALVED (host-scaled
                    # weights), so sigma(x) = (tanh(x/2)+1)/2 makes all four
                    # chunks a single tanh over the whole 4-bank region.
                    tga = stepp.tile([H, 4 * BF], act_dt, tag="tga")
                    nc.scalar.activation(tga, gates[:, 0 : 4 * BF], AF.Tanh)
                    m1 = stepp.tile([H, BF], act_dt, tag="m1")
                    # m1 = (tanh(f/2)+1) * (c0/2)  (c0 pre-halved on host)
                    nc.vector.scalar_tensor_tensor(
                        m1, tga[:, BF : 2 * BF], 1.0, st["c0g"],
                        mybir.AluOpType.add, mybir.AluOpType.mult,
                    )
                    m2 = stepp.tile([H, BF], act_dt, tag="m2")
                    # m2' = (tanh(i/2)+1) * tanh(g)   (true sig_i*tanh_g * 2)
                    nc.vector.scalar_tensor_tensor(
                        m2, tga[:, 0:BF], 1.0, tga[:, 3 * BF : 4 * BF],
                        mybir.AluOpType.add, mybir.AluOpType.mult,
                    )
                    # c_new = m1 + m2'/2
                    nc.vector.scalar_tensor_tensor(
                        cn_out, m2, 0.5, m1,
                        mybir.AluOpType.mult, mybir.AluOpType.add,
                    )
                    st["_sig"] = tga
                    # -Wc accums for t+1 (tanh above was the single reader)
                    if t < T - 1:
                        for c in range(3):
                            nc.tensor.matmul(
                                gates[:, c * BF : (c + 1) * BF],
                                w4_sb[:, G4 + c * H : G4 + (c + 1) * H],
                                prev_r,
                                start=False,
                                stop=(t + 1 == T - 1),
                                skip_group_check=True,
                            )
                    st["_prev_r"] = prev_r
                    return
                tg = stepp.tile([H, BF], act_dt, tag="tg")
                nc.scalar.activation(tg, gdst, AF.Tanh)
                sig = stepp.tile([H, 3 * BF], act_dt, tag="sig")
                if o["sigma_split"]:
                    nc.scalar.activation(sig[:, 0 : 2 * BF], gates[:, 0 : 2 * BF], AF.Sigmoid)
                    nc.scalar.activation(sig[:, 2 * BF : 3 * BF], gates[:, 2 * BF : 3 * BF], AF.Sigmoid)
                elif o["sigma_late"]:
                    # sigma(o) is deferred to step_fin: off the critical chain
                    nc.scalar.activation(sig[:, 0 : 2 * BF], gates[:, 0 : 2 * BF], AF.Sigmoid)
                else:
                    nc.scalar.activation(sig, gates[:, 0 : 3 * BF], AF.Sigmoid)
                if t < T - 1:
                    # -Wc @ rel_{t-1} for step t+1: rel_{t-1} (prev_r) is final,
                    # and sigma above was the last reader of the i/f banks.
                    # (o-chunk handled in step_fin when sigma_late defers its read)
                    nchunks = 2 if o["sigma_late"] else 3
                    for c in range(nchunks):
                        nc.tensor.matmul(
                            gates[:, c * BF : (c + 1) * BF],
                            w4_sb[:, G4 + c * H : G4 + (c + 1) * H],
                            prev_r,
                            start=False,
                            stop=(t + 1 == T - 1),
                            skip_group_check=True,
                        )
                st["_prev_r"] = prev_r
                m1 = stepp.tile([H, BF], act_dt, tag="m1")
                nc.vector.tensor_mul(m1, sig[:, BF : 2 * BF], st["c0g"])
                m2 = stepp.tile([H, BF], act_dt, tag="m2")
                nc.vector.tensor_mul(m2, sig[:, 0:BF], tg)
                nc.vector.tensor_add(cn_out, m1, m2)
                st["_sig"] = sig
                return

            def step_fin(st, t, tct):
                gates = st["gates"]
                b0 = st["b0"]
                cur_tile = st["ping"][t % 2]
                cur_r = st["pingr"][t % 2]
                sig = st.pop("_sig")
                gdst = gates[:, 3 * BF : 4 * BF]
                if o["tanh_all"]:
                    # hn' = (tanh(o/2)+1) * tanh(c_new); the 1/2 is folded
                    # into W_posT (host-halved)
                    hn_dt = act_dt if act_dt != F32 else F32R
                    hn = stepp.tile([H, BF], hn_dt, tag="hn")
                    nc.vector.scalar_tensor_tensor(
                        hn, sig[:, 2 * BF : 3 * BF], 1.0, tct,
                        mybir.AluOpType.add, mybir.AluOpType.mult,
                    )
                elif o["sigma_late"]:
                    nc.scalar.activation(
                        sig[:, 2 * BF : 3 * BF], gates[:, 2 * BF : 3 * BF], AF.Sigmoid
                    )
                    if t < T - 1:
                        nc.tensor.matmul(
                            gates[:, 2 * BF : 3 * BF],
                            w4_sb[:, G4 + 2 * H : G4 + 3 * H],
                            st["_prev_r"],
                            start=False,
                            stop=(t + 1 == T - 1),
                            skip_group_check=True,
                        )
                if not o["tanh_all"]:
                    hn_dt = act_dt if act_dt != F32 else F32R
                    hn = stepp.tile([H, BF], hn_dt, tag="hn")
                    nc.vector.tensor_mul(hn, sig[:, 2 * BF : 3 * BF], tct)
                relp = gates[0:2, 3 * BF : 4 * BF]
                nc.tensor.matmul(
                    relp, wpos_sb, hn, start=True, stop=True,
                    skip_group_check=True,
                )
                if o["fuse_relr"]:
                    # f32r is IEEE-layout rounded fp32: write one f32r tile,
                    # matmuls read it natively, the output DMA reads it as f32
                    nc.vector.tensor_scalar_add(cur_r, relp, bpos_sb)
                    nc.sync.dma_start(
                        relsT_d[t, :, b0 : b0 + BF], cur_r.bitcast(F32)
                    )
                else:
                    nc.vector.tensor_scalar_add(cur_tile, relp, bpos_sb)
                    nc.vector.tensor_copy(cur_r, cur_tile)
                    nc.sync.dma_start(relsT_d[t, :, b0 : b0 + BF], cur_tile)

            for pair in range((ng + 1) // 2):
                ga = 2 * pair
                sts = [setup(ga)]
                if ga + 1 < ng:
                    sts.append(setup(ga + 1))
                for t in range(T):
                    if o["pair_tanh"]:
                        npair = len(sts)
                        cn2 = stepp.tile([H, npair * BF], act_dt, tag="cn2")
                        for i, st in enumerate(sts):
                            step(st, t, cn2[:, i * BF : (i + 1) * BF])
                        tct2 = stepp.tile([H, npair * BF], act_dt, tag="tct2")
                        nc.scalar.activation(tct2, cn2, AF.Tanh)
                        for i, st in enumerate(sts):
                            step_fin(st, t, tct2[:, i * BF : (i + 1) * BF])
                    else:
                        for st in sts:
                            cn = stepp.tile([H, BF], act_dt, tag="cn")
                            step(st, t, cn)
                            tct = stepp.tile([H, BF], act_dt, tag="tct")
                            nc.scalar.activation(tct, cn, AF.Tanh)
                            step_fin(st, t, tct)
    nc.finalize()
    return nc


_CACHE = {}


def _get_nc():
    if "nc" not in _CACHE:
        _CACHE["nc"] = build_nc()
    return _CACHE["nc"]


def make_weights(W_ih, W_hh, b_ih, b_hh, W_emb, b_emb, W_pos, b_pos):
    """Host-side precompute of the derived weight tensors (fp32 numpy)."""
    f = np.float32
    W_ih, W_hh, b_ih, b_hh = f(W_ih), f(W_hh), f(b_ih), f(b_hh)
    W_emb, b_emb, W_pos, b_pos = f(W_emb), f(b_emb), f(W_pos), f(b_pos)
    # psum chunk order [i|f|o|g] over the 4H dim
    perm = np.concatenate(
        [np.arange(0, 256), np.arange(384, 512), np.arange(256, 384)]
    )
    Wc = (W_ih @ W_emb[:, :2])[perm]  # [512, 2]
    wz = (W_ih @ W_emb[:, 2])[perm]  # [512]
    cvec = (b_ih + b_hh + W_ih @ b_emb)[perm]  # [512]
    WhhT = np.ascontiguousarray(W_hh[perm].T)  # [128, 512]
    x, y = Wc[:, 0], Wc[:, 1]
    Wp = np.stack([x, y])  # +Wc: applied to rel_{t-1}
    Wn = np.stack([-x, -y])  # -Wc: applied to rel_{t-2}
    W4 = np.ascontiguousarray(np.concatenate([Wp, Wn], axis=1))  # [2, 1024]
    init2 = np.ascontiguousarray(np.stack([wz, cvec]))  # [2, 512]
    WposT = np.ascontiguousarray(W_pos.T)  # [128, 2]
    if OPTS.get("tanh_all"):
        # sigma(x) = (tanh(x/2)+1)/2: accumulate the i/f/o gate chunks
        # (permuted cols [0:384]) pre-halved; g chunk stays unscaled
        WhhT = WhhT.copy(); WhhT[:, 0:384] *= 0.5
        init2 = init2.copy(); init2[:, 0:384] *= 0.5
        W4 = W4.copy(); W4[:, 0:384] *= 0.5; W4[:, 512:896] *= 0.5
        WposT = WposT * 0.5
    bpos = np.ascontiguousarray(b_pos.reshape(2, 1))
    return dict(WhhT=WhhT, W4=W4, init2=init2, WposT=WposT, bpos=bpos)


def kernel(
    last_pos,
    last_pos_rel,
    h0,
    c0,
    last_speed_pos_rel,
    last_speed_abs_pos,
    seq_len,
    W_ih,
    W_hh,
    b_ih,
    b_hh,
    W_emb,
    b_emb,
    W_pos,
    b_pos,
):
    assert int(seq_len) == T
    f = lambda x: np.asarray(x, dtype=np.float32)
    h0f = f(h0)[0]  # [B, H]
    c0f = f(c0)[0]
    lprel = f(last_pos_rel)
    speed = f(last_speed_pos_rel)
    w = make_weights(W_ih, W_hh, b_ih, b_hh, W_emb, b_emb, W_pos, b_pos)

    in_maps = []
    for c in range(NCORES):
        s = slice(c * BC, (c + 1) * BC)
        in_maps.append(
            dict(
                h0T=np.ascontiguousarray(h0f[s].T),
                c0T=np.ascontiguousarray(
                    c0f[s].T * (0.5 if OPTS.get("tanh_all") else 1.0)
                ),
                speed=np.ascontiguousarray(speed[s].reshape(1, BC)),
                lprelT=np.ascontiguousarray(lprel[s].T),
                **w,
            )
        )
    res = run_bass_kernel_spmd(_get_nc(), in_maps, core_ids=list(range(NCORES)))
    rels = np.concatenate(
        [np.asarray(r["relsT"]).transpose(0, 2, 1) for r in res.results], axis=1
    )
    return rels, np.asarray(h0, dtype=np.float32)[0:1]
